# revision 71
# baseline (speedup 1.0000x reference)
"""Trainium2 Bass kernel for nn_HGAT (B=65536, H=256, C=3, 3 layers).

The reference HGAT collapses algebraically: the p<-attend(xx) stage has
key length 1, so its softmax is identically 1 and p stays of the form
alpha*p0[c] + u[b].  The whole network then reduces, per sample, to

    S   = x @ G                      (3 attention-logit drivers)
    t_l = S[:,l] + sum_{j<l} w_j.m[j][l];  w_l = softmax(leaky(t_l + kc_l))
    y   = x @ A + [w1 w2 w3] @ Bm    (constants folded into Bm)
    out[b,c] = sigmoid(W2.tanh(y + d_c) + b2)

with d_c tiny, so tanh(y+d_c) = t + d_c*(1-t^2), t = tanh(y), giving
logit_c = W2.t - (W2*d_c).t^2 + const_c.

Device layout: y^T (hidden on partitions, samples on free).  8 cores x
8192 samples, 4 pipelined superchunks of 2048 per core, all bf16
matmuls.  Per superchunk: per-group score matmuls, ONE wide softmax
chain on (128,16,3) tiles (executed mid-PREVIOUS-superchunk so the
boundary transposes never wait on it; the layer-0->1 attention dot is
fused onto the unnormalized exponentials to hide per-op semaphore
latency), per-group PE transposes of w staged half-superchunk in PSUM
+ wide DVE copies, then 4 PSUM chunks of 512 in weight-shared pairs:
A-matmuls + Bm-accumulate, tanh, square (DVE), class-logit matmuls.
BOTH logit pairs and the output DMA are deferred to the top of the
next superchunk as chain-latency filler; the last superchunk drains
per-chunk with per-kk-half tanh/square ops and a split output DMA so
logit matmuls and DMA completion overlap the epilogue.  x arrives host-pre-tiled for contiguous 8KB-per-partition
DMA descriptors.  Sigmoid+bias run on the host.
"""

import numpy as np
import ml_dtypes

import concourse.bass as bass
import concourse.bacc as bacc
import concourse.mybir as mybir
from concourse.tile import TileContext
from concourse.bass_utils import run_bass_kernel_spmd

H, C, NL = 256, 3, 3
B = 65536
NCORES = 8
BPC = B // NCORES          # 8192 samples per core
NB = 512                   # samples per PSUM chunk
SC = 2048                  # samples per superchunk
NSC = BPC // SC            # 4
NG = SC // 128             # 16 sample groups per superchunk
NCH = SC // NB             # 4 chunks per superchunk
GPC = NB // 128            # 4 groups per chunk
BF16 = mybir.dt.bfloat16
F32 = mybir.dt.float32
bf16 = ml_dtypes.bfloat16

AF = mybir.ActivationFunctionType
ALU = mybir.AluOpType
AX = mybir.AxisListType


# ----------------------------------------------------------------------
# Host-side precompute (float64): collapse the network.
# ----------------------------------------------------------------------
def _precompute(inp):
    f64 = {k: np.asarray(v, np.float64) for k, v in inp.items()}
    emb, W_rel, b_rel = f64["emb"], f64["W_rel"], f64["b_rel"]
    Wq, bq, Wk, bk = f64["Wq"], f64["bq"], f64["Wk"], f64["bk"]
    Wv, bv, Ws, bs = f64["Wv"], f64["bv"], f64["Ws"], f64["bs"]
    W1, b1, W2, b2 = f64["W1"], f64["b1"], f64["W2"], f64["b2"]

    p0 = emb @ W_rel + b_rel
    Xm, Um = np.eye(H), np.zeros((H, H))
    xc, uc = np.zeros(H), np.zeros(H)
    XW = [None] * NL
    UW = [None] * NL
    alpha = 1.0
    G = np.zeros((H, NL))
    e = np.zeros(NL)
    m = [[None] * NL for _ in range(NL)]
    sk = np.zeros((NL, C))

    for l in range(NL):
        Wq1, bq1 = Wq[l, 0], bq[l, 0]
        Wk1, bk1 = Wk[l, 0], bk[l, 0]
        Wv1, bv1 = Wv[l, 0], bv[l, 0]
        wsq, wsk_ = Ws[l, 0][:H], Ws[l, 0][H:]
        Wv2, bv2 = Wv[l, 1], bv[l, 1]

        aQ, cQ = Wq1 @ wsq, bq1 @ wsq
        aK, cK = Wk1 @ wsk_, bk1 @ wsk_
        ct = cQ + cK + bs[l, 0]
        kap = p0 @ aK
        nu = p0 @ Wv1

        G[:, l] = Xm @ aQ + Um @ aK
        e[l] = xc @ aQ + uc @ aK + ct
        for j in range(l):
            m[j][l] = XW[j] @ aQ + UW[j] @ aK
        sk[l] = alpha * kap

        Xm2 = 2 * Xm + Um @ Wv1
        xc2 = 2 * xc + uc @ Wv1 + bv1
        XW2 = [2 * XW[j] + UW[j] @ Wv1 if XW[j] is not None else None
               for j in range(NL)]
        XW2[l] = alpha * nu
        Um2 = 2 * Um + Xm2 @ Wv2
        uc2 = 2 * uc + xc2 @ Wv2 + bv2
        UW2 = [
            (XW2[j] @ Wv2 + (2 * UW[j] if UW[j] is not None else 0.0))
            if XW2[j] is not None else None
            for j in range(NL)
        ]
        Xm, Um, xc, uc, XW, UW = Xm2, Um2, xc2, uc2, XW2, UW2
        alpha *= 2

    A = Um @ W1
    Bm = np.stack([UW[j] @ W1 for j in range(NL)]).reshape(NL * C, H)
    Bm[0:C] += uc @ W1          # fold constant via sum_c w_l = 1
    d = alpha * (p0 @ W1) + b1  # (C,H)
    W2v, b2v = W2[:, 0], b2[0]
    rho = W2v[None, :] * d      # (C,H)
    return dict(G=G, e=e, m=m, sk=sk, A=A, Bm=Bm, rho=rho,
                W2=W2v, kappa=rho.sum(1) + b2v)


NB16 = 512 + 6 + 12 + 256 + 128   # A | G | EtEp | Bm | ID cols
NF32 = NL * C + C + 2 * C         # KC | M12 | M1323 cols


def _device_consts(P):
    A = np.asarray(P["A"])
    cb = np.zeros((128, NB16), bf16)
    o = 0
    cb[:, o:o + 512] = A.reshape(2, 128, 2, 128).transpose(
        1, 0, 2, 3).reshape(128, 512).astype(bf16); o += 512
    cb[:, o:o + 6] = P["G"].reshape(2, 128, C).transpose(
        1, 0, 2).reshape(128, 6).astype(bf16); o += 6
    W2h = np.asarray(P["W2"]).reshape(2, 128)
    rhoh = np.asarray(P["rho"]).T.reshape(2, 128, C)
    etep = np.empty((128, 2, 2, C), np.float64)
    etep[:, :, 0, :] = np.repeat(W2h.T[:, :, None], C, axis=2)
    etep[:, :, 1, :] = -rhoh.transpose(1, 0, 2)
    cb[:, o:o + 12] = etep.reshape(128, 12).astype(bf16); o += 12
    cb[0:NL * C, o:o + 256] = np.asarray(P["Bm"]).astype(bf16); o += 256
    cb[:, o:o + 128] = np.eye(128, dtype=np.float32).astype(bf16); o += 128
    assert o == NB16

    cf = np.zeros((128, NF32), np.float32)
    o = 0
    kc = (P["sk"] + P["e"][:, None]).reshape(1, NL * C)
    cf[:, o:o + NL * C] = kc; o += NL * C
    cf[:, o:o + C] = P["m"][0][1].reshape(1, C); o += C
    cf[:, o:o + 2 * C] = np.concatenate(
        [P["m"][0][2], P["m"][1][2]]).reshape(1, 2 * C); o += 2 * C
    assert o == NF32
    return {"CB16": cb, "CF32": cf}


# ----------------------------------------------------------------------
# Bass program (built once per process)
# ----------------------------------------------------------------------
def _build_nc():
    nc = bacc.Bacc()
    xT = nc.dram_tensor("xT", (128, NSC * 2 * SC), BF16,
                        kind="ExternalInput")
    CB16 = nc.dram_tensor("CB16", (128, NB16), BF16, kind="ExternalInput")
    CF32 = nc.dram_tensor("CF32", (128, NF32), F32, kind="ExternalInput")
    outT = nc.dram_tensor("outT", (C, BPC), F32, kind="ExternalOutput")

    with TileContext(nc) as tc:
        with (
            tc.tile_pool(name="consts", bufs=1) as cpool,
            tc.tile_pool(name="xt", bufs=3) as xtp,
            tc.tile_pool(name="sv", bufs=3) as svp,
            tc.tile_pool(name="chain", bufs=4) as chp,
            tc.tile_pool(name="wsb", bufs=3) as wsp,
            tc.tile_pool(name="wt", bufs=2) as wtp,
            tc.tile_pool(name="tsq", bufs=5) as tsp,
            tc.tile_pool(name="lout", bufs=2) as lop,
            tc.tile_pool(name="ps", bufs=1, space="PSUM") as psp,
            tc.tile_pool(name="py", bufs=1, space="PSUM") as pyp,
            tc.tile_pool(name="pl", bufs=1, space="PSUM") as plp,
            tc.tile_pool(name="pwt", bufs=1, space="PSUM") as pwtp,
        ):
            cb_sb = cpool.tile([128, NB16], BF16)
            nc.sync.dma_start(out=cb_sb, in_=CB16[:, :])
            cf_sb = cpool.tile([128, NF32], F32)
            nc.sync.dma_start(out=cf_sb, in_=CF32[:, :])
            A_sb = cb_sb[:, 0:512].rearrange(
                "p (kk mm n) -> p kk mm n", kk=2, mm=2)
            G_sb = cb_sb[:, 512:518].rearrange("p (k c) -> p k c", c=C)
            EtEp = cb_sb[:, 518:530].rearrange(
                "p (k e c) -> p k e c", k=2, e=2)
            Bm_sb = cb_sb[0:NL * C, 530:786].rearrange(
                "w (mm n) -> w mm n", mm=2)
            ID_sb = cb_sb[:, 786:914]
            KC = cf_sb[:, 0:NL * C].rearrange("p (l c) -> p l c", c=C)
            M12_sb = cf_sb[:, NL * C:NL * C + C]
            M1323_sb = cf_sb[:, NL * C + C:NL * C + 3 * C]

            # First-touch the chain consts on DVE (S3S3D3_TT walrus struct
            # has a single sync-wait slot).
            warm = cpool.tile([128, 1], F32)
            nc.vector.tensor_copy(out=warm, in_=cf_sb[:, 0:1])

            def softmax3(sc_in, wout, ng, dot=None):
                """leaky-relu + softmax over last axis (3) of (128,NG,3).
                If dot=(mv, tout): also computes tout = (softmax . mv) via
                the unnormalized exponentials, ordered so consecutive DVE
                ops depend >=2 slots back (hides the ~190ns @complete
                semaphore latency of back-to-back dependent ops)."""
                lr = chp.tile([128, ng, C], F32, tag="lr")
                nc.vector.scalar_tensor_tensor(
                    out=lr, in0=sc_in, scalar=0.2, in1=sc_in,
                    op0=ALU.mult, op1=ALU.max)
                esc = chp.tile([128, ng, C], F32, tag="esc")
                nc.scalar.activation(out=esc, in_=lr, func=AF.Exp)
                red = chp.tile([128, ng], F32, tag="red")
                nc.vector.reduce_sum(out=red, in_=esc, axis=AX.X)
                if dot is not None:
                    mv, tout = dot
                    um = chp.tile([128, ng, C], F32, tag="tmp")
                    nc.vector.tensor_tensor(
                        out=um, in0=esc,
                        in1=mv.rearrange("p c -> p () c").broadcast_to(
                            (128, ng, C)),
                        op=ALU.mult)
                    ur = chp.tile([128, ng], F32, tag="t1")
                    nc.vector.reduce_sum(out=ur, in_=um, axis=AX.X)
                rec = chp.tile([128, ng], F32, tag="rec")
                nc.vector.reciprocal(out=rec, in_=red)
                nc.vector.tensor_tensor(
                    out=wout, in0=esc,
                    in1=rec.broadcast_to((128, ng, C)),
                    op=ALU.mult)
                if dot is not None:
                    nc.vector.tensor_tensor(out=tout, in0=ur, in1=rec,
                                            op=ALU.mult)

            def emit_xt_dma(sc):
                """x arrives host-pre-tiled: per superchunk each partition
                reads one contiguous 8KB run; issued on the ACT HWDGE
                queue so it overlaps Sync's const/output DMAs."""
                xt = xtp.tile([128, 2, SC], BF16)
                nc.sync.dma_start(
                    out=xt,
                    in_=xT[:, sc * 2 * SC:(sc + 1) * 2 * SC].rearrange(
                        "p (k b) -> p k b", k=2))
                return xt

            def emit_scores(xt):
                """Per-group score matmuls for a superchunk (PE)."""
                ps = psp.tile([128, NG, C], F32)
                for g in range(NG):
                    for kk in (0, 1):
                        nc.tensor.matmul(
                            ps[:, g, :],
                            lhsT=xt[:, kk, g * 128:(g + 1) * 128],
                            rhs=G_sb[:, kk, :],
                            start=(kk == 0), stop=(kk == 1))
                return ps

            def emit_chain(ps, w_sb, g0=0, ng=NG):
                """Softmax chain for groups [g0, g0+ng) into w_sb.  Reads
                the score PSUM directly: since the chain runs a full block
                before the ps buffer's next writer, no staging copy is
                needed, and dropping it removes one serial op + semaphore
                hop from the chain spine."""
                sv = ps[:, g0:g0 + ng, :]
                wv = w_sb[:, g0:g0 + ng, :]

                sc0 = chp.tile([128, ng, C], F32, tag="sc")
                nc.vector.tensor_tensor(
                    out=sc0,
                    in0=sv[:, :, 0:1].broadcast_to((128, ng, C)),
                    in1=KC[:, 0:1, :].broadcast_to((128, ng, C)),
                    op=ALU.add)
                t1 = chp.tile([128, ng], F32, tag="t1o")
                softmax3(sc0, wv[:, :, 0:C], ng, dot=(M12_sb, t1))

                t1b = chp.tile([128, ng], F32, tag="t1b")
                nc.vector.tensor_tensor(
                    out=t1b, in0=t1,
                    in1=sv[:, :, 1:2].rearrange("p j one -> p (j one)"),
                    op=ALU.add)
                sc1 = chp.tile([128, ng, C], F32, tag="sc")
                nc.vector.tensor_tensor(
                    out=sc1,
                    in0=t1b.broadcast_to((128, ng, C)),
                    in1=KC[:, 1:2, :].broadcast_to((128, ng, C)),
                    op=ALU.add)
                softmax3(sc1, wv[:, :, C:2 * C], ng)

                tmp6 = chp.tile([128, ng, 2 * C], F32, tag="tmp6")
                nc.vector.tensor_tensor(
                    out=tmp6, in0=wv[:, :, 0:2 * C],
                    in1=M1323_sb.rearrange("p c -> p () c").broadcast_to(
                        (128, ng, 2 * C)),
                    op=ALU.mult)
                t2 = chp.tile([128, ng], F32, tag="t1")
                nc.vector.reduce_sum(out=t2, in_=tmp6, axis=AX.X)
                t2b = chp.tile([128, ng], F32, tag="t1b")
                nc.vector.tensor_tensor(
                    out=t2b, in0=t2,
                    in1=sv[:, :, 2:3].rearrange("p j one -> p (j one)"),
                    op=ALU.add)
                sc2 = chp.tile([128, ng, C], F32, tag="sc")
                nc.vector.tensor_tensor(
                    out=sc2,
                    in0=t2b.broadcast_to((128, ng, C)),
                    in1=KC[:, 2:3, :].broadcast_to((128, ng, C)),
                    op=ALU.add)
                softmax3(sc2, wv[:, :, 2 * C:3 * C], ng)

            def emit_chain_full(ps, split):
                """split=True: two half-group passes so the first wt-half
                unblocks after ~half the chain's serial spine (used for
                the latency-critical chains 0 and 1)."""
                w_sb = wsp.tile([128, NG, NL * C], BF16)
                if split:
                    emit_chain(ps, w_sb, 0, NG // 2)
                    emit_chain(ps, w_sb, NG // 2, NG // 2)
                else:
                    emit_chain(ps, w_sb)
                return w_sb

            def emit_wt_half(w_sb, wt, h):
                """Transpose the 8 groups feeding chunks 2h, 2h+1."""
                pwt = pwtp.tile([NL * C, NG // 2, 128], BF16)
                for j in range(NG // 2):
                    nc.tensor.transpose(
                        pwt[:, j, :], w_sb[:, h * 8 + j, :], ID_sb)
                nc.vector.tensor_copy(
                    out=wt[:, h * 8:(h + 1) * 8, :], in_=pwt)

            def emit_a_pair(xt, c0):
                """A matmuls for chunks c0, c0+1; shared-weight matmuls
                adjacent."""
                pya = pyp.tile([128, 2, NB], F32, tag="pya")
                pyb = pyp.tile([128, 2, NB], F32, tag="pyb")
                for mm in (0, 1):
                    for kk in (0, 1):
                        for py, c in ((pya, c0), (pyb, c0 + 1)):
                            nc.tensor.matmul(
                                py[:, mm, :], lhsT=A_sb[:, kk, mm, :],
                                rhs=xt[:, kk, c * NB:(c + 1) * NB],
                                start=(kk == 0), stop=False)
                return pya, pyb

            def emit_bm_pair(wt, pya, pyb, c0):
                for mm in (0, 1):
                    for py, c in ((pya, c0), (pyb, c0 + 1)):
                        nc.tensor.matmul(
                            py[:, mm, :],
                            lhsT=Bm_sb[:, mm, :],
                            rhs=wt[0:NL * C, c * GPC:(c + 1) * GPC, :],
                            start=False, stop=True)

            def emit_tanh_sq(py, split=False):
                """split=True halves the ops per kk so downstream logit
                matmuls can start after half the tanh (epilogue tail)."""
                t_sb = tsp.tile([128, 2, NB], BF16, tag="t")
                p2_sb = tsp.tile([128, 2, NB], BF16, tag="p2")
                if split:
                    for kk in (0, 1):
                        nc.scalar.activation(
                            out=t_sb[:, kk, :], in_=py[:, kk, :],
                            func=AF.Tanh)
                    for kk in (0, 1):
                        nc.vector.tensor_tensor(
                            out=p2_sb[:, kk, :], in0=t_sb[:, kk, :],
                            in1=t_sb[:, kk, :], op=ALU.mult)
                else:
                    nc.scalar.activation(
                        out=t_sb.rearrange("p k b -> p (k b)"),
                        in_=py.rearrange("p k b -> p (k b)"),
                        func=AF.Tanh)
                    nc.vector.tensor_tensor(
                        out=p2_sb.rearrange("p k b -> p (k b)"),
                        in0=t_sb.rearrange("p k b -> p (k b)"),
                        in1=t_sb.rearrange("p k b -> p (k b)"),
                        op=ALU.mult)
                return t_sb, p2_sb

            def emit_pl_pair(st, c0):
                """Class-logit matmuls for chunks c0, c0+1, weight-shared."""
                ts, t2s, L_sb = st["ts"], st["t2s"], st["L"]
                pla = plp.tile([C, NB], F32, tag="pla")
                plb = plp.tile([C, NB], F32, tag="plb")
                pls = [pla, plb]
                rhs = {0: ts, 1: t2s}
                first, last = (0, 0), (1, 1)
                for e in (0, 1):
                    for kk in (0, 1):
                        for i, c in enumerate((c0, c0 + 1)):
                            nc.tensor.matmul(
                                pls[i], lhsT=EtEp[:, kk, e, :],
                                rhs=rhs[e][c][:, kk, :],
                                start=(e, kk) == first,
                                stop=(e, kk) == last)
                for i, c in enumerate((c0, c0 + 1)):
                    nc.vector.tensor_copy(
                        out=L_sb[:, c * NB:(c + 1) * NB], in_=pls[i])

            def emit_pl_one(st, c):
                """Single-chunk class-logit matmuls (epilogue tail)."""
                ts, t2s, L_sb = st["ts"], st["t2s"], st["L"]
                plx = plp.tile([C, NB], F32, tag="pla" if c % 2 == 0
                               else "plb")
                rhs = {0: ts, 1: t2s}
                for e in (0, 1):
                    for kk in (0, 1):
                        nc.tensor.matmul(
                            plx, lhsT=EtEp[:, kk, e, :],
                            rhs=rhs[e][c][:, kk, :],
                            start=(e, kk) == (0, 0),
                            stop=(e, kk) == (1, 1))
                nc.vector.tensor_copy(
                    out=L_sb[:, c * NB:(c + 1) * NB], in_=plx)

            def flush_tail(st):
                """BOTH pl pairs + output DMA of the previous superchunk,
                emitted at the top of the next one: ~6us of chain-
                independent PE filler for the softmax-chain latency."""
                emit_pl_pair(st, 0)
                emit_pl_pair(st, 2)
                nc.sync.dma_start(
                    out=outT[:, st["sc"] * SC:(st["sc"] + 1) * SC],
                    in_=st["L"])

            # prologue: prefetch two superchunks of x, scores for sc 0
            xts, pss = [None] * NSC, [None] * NSC
            ws = [None] * NSC
            xts[0] = emit_xt_dma(0)
            xts[1] = emit_xt_dma(1)
            pss[0] = emit_scores(xts[0])
            ws[0] = emit_chain_full(pss[0], False)

            prev = None
            for sc in range(NSC):
                xt, w_sb = xts[sc], ws[sc]
                if prev is not None:
                    flush_tail(prev)
                if sc + 2 < NSC:
                    xts[sc + 2] = emit_xt_dma(sc + 2)
                if sc + 1 < NSC:
                    pss[sc + 1] = emit_scores(xts[sc + 1])

                wt = wtp.tile([NL * C, NG, 128], BF16)
                L_sb = lop.tile([C, SC], F32)
                st = {"sc": sc, "L": L_sb,
                      "ts": [None] * NCH, "t2s": [None] * NCH}

                emit_wt_half(w_sb, wt, 0)
                pya, pyb = emit_a_pair(xt, 0)
                emit_bm_pair(wt, pya, pyb, 0)
                st["ts"][0], st["t2s"][0] = emit_tanh_sq(pya)
                st["ts"][1], st["t2s"][1] = emit_tanh_sq(pyb)
                emit_wt_half(w_sb, wt, 1)
                if sc + 1 < NSC:
                    # next superchunk's chain runs mid-block so the next
                    # boundary's transposes never wait on it
                    ws[sc + 1] = emit_chain_full(pss[sc + 1], False)
                pyc, pyd = emit_a_pair(xt, 2)
                emit_bm_pair(wt, pyc, pyd, 2)
                if sc == NSC - 1:
                    emit_pl_pair(st, 0)
                    st["ts"][2], st["t2s"][2] = emit_tanh_sq(pyc,
                                                            split=True)
                    emit_pl_one(st, 2)
                    nc.sync.dma_start(
                        out=outT[:, sc * SC:sc * SC + 3 * NB],
                        in_=st["L"][:, 0:3 * NB])
                    st["ts"][3], st["t2s"][3] = emit_tanh_sq(pyd,
                                                            split=True)
                    emit_pl_one(st, 3)
                    nc.sync.dma_start(
                        out=outT[:, sc * SC + 3 * NB:(sc + 1) * SC],
                        in_=st["L"][:, 3 * NB:SC])
                else:
                    st["ts"][2], st["t2s"][2] = emit_tanh_sq(pyc)
                    st["ts"][3], st["t2s"][3] = emit_tanh_sq(pyd)
                    prev = st
    nc.finalize()
    return nc


_NC_CACHE = None


def _get_nc():
    global _NC_CACHE
    if _NC_CACHE is None:
        _NC_CACHE = _build_nc()
    return _NC_CACHE


def _run(inputs, trace=False):
    P = _precompute(inputs)
    cst = _device_consts(P)
    x = np.asarray(inputs["x"], np.float32)
    xTb = np.ascontiguousarray(x.astype(bf16).T)      # (256, B)
    nc = _get_nc()
    in_maps = []
    for c in range(NCORES):
        m = dict(cst)
        xc = xTb[:, c * BPC:(c + 1) * BPC]            # (256, BPC)
        xc = xc.reshape(2, 128, NSC, SC).transpose(1, 2, 0, 3)
        m["xT"] = np.ascontiguousarray(xc.reshape(128, NSC * 2 * SC))
        in_maps.append(m)
    res = run_bass_kernel_spmd(nc, in_maps, list(range(NCORES)),
                               trace=trace)
    kap = np.asarray(P["kappa"], np.float32)
    out = np.empty((B, C), np.float32)
    for c in range(NCORES):
        L = res.results[c]["outT"].T + kap[None, :]
        out[c * BPC:(c + 1) * BPC] = 1.0 / (1.0 + np.exp(-L))
    return out, res


def kernel(**inputs):
    out, _ = _run(inputs, trace=False)
    return out


# revision 72
# speedup vs baseline: 1.0255x; 1.0255x over previous
"""Trainium2 Bass kernel for nn_HGAT (B=65536, H=256, C=3, 3 layers).

The reference HGAT collapses algebraically: the p<-attend(xx) stage has
key length 1, so its softmax is identically 1 and p stays of the form
alpha*p0[c] + u[b].  The whole network then reduces, per sample, to

    S   = x @ G                      (3 attention-logit drivers)
    t_l = S[:,l] + sum_{j<l} w_j.m[j][l];  w_l = softmax(leaky(t_l + kc_l))
    y   = x @ A + [w1 w2 w3] @ Bm    (constants folded into Bm)
    out[b,c] = sigmoid(W2.tanh(y + d_c) + b2)

with d_c tiny, so tanh(y+d_c) = t + d_c*(1-t^2), t = tanh(y), giving
logit_c = W2.t - (W2*d_c).t^2 + const_c.

Device layout: y^T (hidden on partitions, samples on free).  8 cores x
8192 samples, 4 pipelined superchunks of 2048 per core, all bf16
matmuls.  Per superchunk: per-group score matmuls, ONE wide softmax
chain on (128,16,3) tiles (executed mid-PREVIOUS-superchunk so the
boundary transposes never wait on it; the layer-0->1 attention dot is
fused onto the unnormalized exponentials to hide per-op semaphore
latency), per-group PE transposes of w staged half-superchunk in PSUM
+ wide DVE copies, then 4 PSUM chunks of 512 in weight-shared pairs:
A-matmuls + Bm-accumulate, tanh, square (DVE), class-logit matmuls.
BOTH logit pairs and the output DMA are deferred to the top of the
next superchunk as chain-latency filler; the last superchunk drains
per-chunk with per-kk-half tanh/square ops and a split output DMA so
logit matmuls and DMA completion overlap the epilogue.  x arrives host-pre-tiled for contiguous 8KB-per-partition
DMA descriptors.  Sigmoid+bias run on the host.
"""

import numpy as np
import ml_dtypes

import concourse.bass as bass
import concourse.bacc as bacc
import concourse.mybir as mybir
from concourse.tile import TileContext
from concourse.bass_utils import run_bass_kernel_spmd

H, C, NL = 256, 3, 3
B = 65536
NCORES = 8
BPC = B // NCORES          # 8192 samples per core
NB = 512                   # samples per PSUM chunk
SC = 2048                  # samples per superchunk
NSC = BPC // SC            # 4
NG = SC // 128             # 16 sample groups per superchunk
NCH = SC // NB             # 4 chunks per superchunk
GPC = NB // 128            # 4 groups per chunk
BF16 = mybir.dt.bfloat16
F32 = mybir.dt.float32
bf16 = ml_dtypes.bfloat16

AF = mybir.ActivationFunctionType
ALU = mybir.AluOpType
AX = mybir.AxisListType


# ----------------------------------------------------------------------
# Host-side precompute (float64): collapse the network.
# ----------------------------------------------------------------------
def _precompute(inp):
    f64 = {k: np.asarray(v, np.float64) for k, v in inp.items()}
    emb, W_rel, b_rel = f64["emb"], f64["W_rel"], f64["b_rel"]
    Wq, bq, Wk, bk = f64["Wq"], f64["bq"], f64["Wk"], f64["bk"]
    Wv, bv, Ws, bs = f64["Wv"], f64["bv"], f64["Ws"], f64["bs"]
    W1, b1, W2, b2 = f64["W1"], f64["b1"], f64["W2"], f64["b2"]

    p0 = emb @ W_rel + b_rel
    Xm, Um = np.eye(H), np.zeros((H, H))
    xc, uc = np.zeros(H), np.zeros(H)
    XW = [None] * NL
    UW = [None] * NL
    alpha = 1.0
    G = np.zeros((H, NL))
    e = np.zeros(NL)
    m = [[None] * NL for _ in range(NL)]
    sk = np.zeros((NL, C))

    for l in range(NL):
        Wq1, bq1 = Wq[l, 0], bq[l, 0]
        Wk1, bk1 = Wk[l, 0], bk[l, 0]
        Wv1, bv1 = Wv[l, 0], bv[l, 0]
        wsq, wsk_ = Ws[l, 0][:H], Ws[l, 0][H:]
        Wv2, bv2 = Wv[l, 1], bv[l, 1]

        aQ, cQ = Wq1 @ wsq, bq1 @ wsq
        aK, cK = Wk1 @ wsk_, bk1 @ wsk_
        ct = cQ + cK + bs[l, 0]
        kap = p0 @ aK
        nu = p0 @ Wv1

        G[:, l] = Xm @ aQ + Um @ aK
        e[l] = xc @ aQ + uc @ aK + ct
        for j in range(l):
            m[j][l] = XW[j] @ aQ + UW[j] @ aK
        sk[l] = alpha * kap

        Xm2 = 2 * Xm + Um @ Wv1
        xc2 = 2 * xc + uc @ Wv1 + bv1
        XW2 = [2 * XW[j] + UW[j] @ Wv1 if XW[j] is not None else None
               for j in range(NL)]
        XW2[l] = alpha * nu
        Um2 = 2 * Um + Xm2 @ Wv2
        uc2 = 2 * uc + xc2 @ Wv2 + bv2
        UW2 = [
            (XW2[j] @ Wv2 + (2 * UW[j] if UW[j] is not None else 0.0))
            if XW2[j] is not None else None
            for j in range(NL)
        ]
        Xm, Um, xc, uc, XW, UW = Xm2, Um2, xc2, uc2, XW2, UW2
        alpha *= 2

    A = Um @ W1
    Bm = np.stack([UW[j] @ W1 for j in range(NL)]).reshape(NL * C, H)
    Bm[0:C] += uc @ W1          # fold constant via sum_c w_l = 1
    d = alpha * (p0 @ W1) + b1  # (C,H)
    W2v, b2v = W2[:, 0], b2[0]
    rho = W2v[None, :] * d      # (C,H)
    return dict(G=G, e=e, m=m, sk=sk, A=A, Bm=Bm, rho=rho,
                W2=W2v, kappa=rho.sum(1) + b2v)


NB16 = 512 + 6 + 12 + 256 + 128   # A | G | EtEp | Bm | ID cols
NF32 = NL * C + C + 2 * C         # KC | M12 | M1323 cols


def _device_consts(P):
    A = np.asarray(P["A"])
    cb = np.zeros((128, NB16), bf16)
    o = 0
    cb[:, o:o + 512] = A.reshape(2, 128, 2, 128).transpose(
        1, 0, 2, 3).reshape(128, 512).astype(bf16); o += 512
    cb[:, o:o + 6] = P["G"].reshape(2, 128, C).transpose(
        1, 0, 2).reshape(128, 6).astype(bf16); o += 6
    W2h = np.asarray(P["W2"]).reshape(2, 128)
    rhoh = np.asarray(P["rho"]).T.reshape(2, 128, C)
    etep = np.empty((128, 2, 2, C), np.float64)
    etep[:, :, 0, :] = np.repeat(W2h.T[:, :, None], C, axis=2)
    etep[:, :, 1, :] = -rhoh.transpose(1, 0, 2)
    cb[:, o:o + 12] = etep.reshape(128, 12).astype(bf16); o += 12
    cb[0:NL * C, o:o + 256] = np.asarray(P["Bm"]).astype(bf16); o += 256
    cb[:, o:o + 128] = np.eye(128, dtype=np.float32).astype(bf16); o += 128
    assert o == NB16

    cf = np.zeros((128, NF32), np.float32)
    o = 0
    kc = (P["sk"] + P["e"][:, None]).reshape(1, NL * C)
    cf[:, o:o + NL * C] = kc; o += NL * C
    cf[:, o:o + C] = P["m"][0][1].reshape(1, C); o += C
    cf[:, o:o + 2 * C] = np.concatenate(
        [P["m"][0][2], P["m"][1][2]]).reshape(1, 2 * C); o += 2 * C
    assert o == NF32
    return {"CB16": cb, "CF32": cf}


# ----------------------------------------------------------------------
# Bass program (built once per process)
# ----------------------------------------------------------------------
def _build_nc():
    nc = bacc.Bacc()
    xT = nc.dram_tensor("xT", (128, NSC * 2 * SC), BF16,
                        kind="ExternalInput")
    CB16 = nc.dram_tensor("CB16", (128, NB16), BF16, kind="ExternalInput")
    CF32 = nc.dram_tensor("CF32", (128, NF32), F32, kind="ExternalInput")
    outT = nc.dram_tensor("outT", (C, BPC), F32, kind="ExternalOutput")

    with TileContext(nc) as tc:
        with (
            tc.tile_pool(name="consts", bufs=1) as cpool,
            tc.tile_pool(name="xt", bufs=3) as xtp,
            tc.tile_pool(name="sv", bufs=3) as svp,
            tc.tile_pool(name="chain", bufs=4) as chp,
            tc.tile_pool(name="wsb", bufs=3) as wsp,
            tc.tile_pool(name="wt", bufs=2) as wtp,
            tc.tile_pool(name="tsq", bufs=5) as tsp,
            tc.tile_pool(name="lout", bufs=2) as lop,
            tc.tile_pool(name="ps", bufs=1, space="PSUM") as psp,
            tc.tile_pool(name="py", bufs=1, space="PSUM") as pyp,
            tc.tile_pool(name="pl", bufs=1, space="PSUM") as plp,
            tc.tile_pool(name="pwt", bufs=1, space="PSUM") as pwtp,
        ):
            cb_sb = cpool.tile([128, NB16], BF16)
            nc.sync.dma_start(out=cb_sb, in_=CB16[:, :])
            cf_sb = cpool.tile([128, NF32], F32)
            nc.sync.dma_start(out=cf_sb, in_=CF32[:, :])
            A_sb = cb_sb[:, 0:512].rearrange(
                "p (kk mm n) -> p kk mm n", kk=2, mm=2)
            G_sb = cb_sb[:, 512:518].rearrange("p (k c) -> p k c", c=C)
            EtEp = cb_sb[:, 518:530].rearrange(
                "p (k e c) -> p k e c", k=2, e=2)
            Bm_sb = cb_sb[0:NL * C, 530:786].rearrange(
                "w (mm n) -> w mm n", mm=2)
            ID_sb = cb_sb[:, 786:914]
            KC = cf_sb[:, 0:NL * C].rearrange("p (l c) -> p l c", c=C)
            M12_sb = cf_sb[:, NL * C:NL * C + C]
            M1323_sb = cf_sb[:, NL * C + C:NL * C + 3 * C]

            # First-touch the chain consts on DVE (S3S3D3_TT walrus struct
            # has a single sync-wait slot).
            warm = cpool.tile([128, 1], F32)
            nc.vector.tensor_copy(out=warm, in_=cf_sb[:, 0:1])

            def softmax3(sc_in, wout, ng, dot=None):
                """leaky-relu + softmax over last axis (3) of (128,NG,3).
                If dot=(mv, tout): also computes tout = (softmax . mv) via
                the unnormalized exponentials, ordered so consecutive DVE
                ops depend >=2 slots back (hides the ~190ns @complete
                semaphore latency of back-to-back dependent ops)."""
                lr = chp.tile([128, ng, C], F32, tag="lr")
                nc.vector.scalar_tensor_tensor(
                    out=lr, in0=sc_in, scalar=0.2, in1=sc_in,
                    op0=ALU.mult, op1=ALU.max)
                esc = chp.tile([128, ng, C], F32, tag="esc")
                nc.scalar.activation(out=esc, in_=lr, func=AF.Exp)
                red = chp.tile([128, ng], F32, tag="red")
                nc.vector.reduce_sum(out=red, in_=esc, axis=AX.X)
                if dot is not None:
                    mv, tout = dot
                    um = chp.tile([128, ng, C], F32, tag="tmp")
                    nc.vector.tensor_tensor(
                        out=um, in0=esc,
                        in1=mv.rearrange("p c -> p () c").broadcast_to(
                            (128, ng, C)),
                        op=ALU.mult)
                    ur = chp.tile([128, ng], F32, tag="t1")
                    nc.vector.reduce_sum(out=ur, in_=um, axis=AX.X)
                rec = chp.tile([128, ng], F32, tag="rec")
                nc.vector.reciprocal(out=rec, in_=red)
                nc.vector.tensor_tensor(
                    out=wout, in0=esc,
                    in1=rec.broadcast_to((128, ng, C)),
                    op=ALU.mult)
                if dot is not None:
                    nc.vector.tensor_tensor(out=tout, in0=ur, in1=rec,
                                            op=ALU.mult)

            def emit_xt_dma(sc):
                """x arrives host-pre-tiled: per superchunk each partition
                reads one contiguous 8KB run; issued on the ACT HWDGE
                queue so it overlaps Sync's const/output DMAs."""
                xt = xtp.tile([128, 2, SC], BF16)
                nc.sync.dma_start(
                    out=xt,
                    in_=xT[:, sc * 2 * SC:(sc + 1) * 2 * SC].rearrange(
                        "p (k b) -> p k b", k=2))
                return xt

            def emit_scores(xt):
                """Per-group score matmuls for a superchunk (PE)."""
                ps = psp.tile([128, NG, C], F32)
                for g in range(NG):
                    for kk in (0, 1):
                        nc.tensor.matmul(
                            ps[:, g, :],
                            lhsT=xt[:, kk, g * 128:(g + 1) * 128],
                            rhs=G_sb[:, kk, :],
                            start=(kk == 0), stop=(kk == 1))
                return ps

            def emit_chain(ps, w_sb, g0=0, ng=NG):
                """Softmax chain for groups [g0, g0+ng) into w_sb."""
                sv = svp.tile([128, ng, C], F32)
                nc.vector.tensor_copy(out=sv, in_=ps[:, g0:g0 + ng, :])
                wv = w_sb[:, g0:g0 + ng, :]

                sc0 = chp.tile([128, ng, C], F32, tag="sc")
                nc.vector.tensor_tensor(
                    out=sc0,
                    in0=sv[:, :, 0:1].broadcast_to((128, ng, C)),
                    in1=KC[:, 0:1, :].broadcast_to((128, ng, C)),
                    op=ALU.add)
                t1 = chp.tile([128, ng], F32, tag="t1o")
                softmax3(sc0, wv[:, :, 0:C], ng, dot=(M12_sb, t1))

                t1b = chp.tile([128, ng], F32, tag="t1b")
                nc.vector.tensor_tensor(
                    out=t1b, in0=t1,
                    in1=sv[:, :, 1:2].rearrange("p j one -> p (j one)"),
                    op=ALU.add)
                sc1 = chp.tile([128, ng, C], F32, tag="sc")
                nc.vector.tensor_tensor(
                    out=sc1,
                    in0=t1b.broadcast_to((128, ng, C)),
                    in1=KC[:, 1:2, :].broadcast_to((128, ng, C)),
                    op=ALU.add)
                softmax3(sc1, wv[:, :, C:2 * C], ng)

                tmp6 = chp.tile([128, ng, 2 * C], F32, tag="tmp6")
                nc.vector.tensor_tensor(
                    out=tmp6, in0=wv[:, :, 0:2 * C],
                    in1=M1323_sb.rearrange("p c -> p () c").broadcast_to(
                        (128, ng, 2 * C)),
                    op=ALU.mult)
                t2 = chp.tile([128, ng], F32, tag="t1")
                nc.vector.reduce_sum(out=t2, in_=tmp6, axis=AX.X)
                t2b = chp.tile([128, ng], F32, tag="t1b")
                nc.vector.tensor_tensor(
                    out=t2b, in0=t2,
                    in1=sv[:, :, 2:3].rearrange("p j one -> p (j one)"),
                    op=ALU.add)
                sc2 = chp.tile([128, ng, C], F32, tag="sc")
                nc.vector.tensor_tensor(
                    out=sc2,
                    in0=t2b.broadcast_to((128, ng, C)),
                    in1=KC[:, 2:3, :].broadcast_to((128, ng, C)),
                    op=ALU.add)
                softmax3(sc2, wv[:, :, 2 * C:3 * C], ng)

            def emit_chain_full(ps, split):
                """split=True: two half-group passes so the first wt-half
                unblocks after ~half the chain's serial spine (used for
                the latency-critical chains 0 and 1)."""
                w_sb = wsp.tile([128, NG, NL * C], BF16)
                if split:
                    emit_chain(ps, w_sb, 0, NG // 2)
                    emit_chain(ps, w_sb, NG // 2, NG // 2)
                else:
                    emit_chain(ps, w_sb)
                return w_sb

            def emit_wt_half(w_sb, wt, h):
                """Transpose the 8 groups feeding chunks 2h, 2h+1."""
                pwt = pwtp.tile([NL * C, NG // 2, 128], BF16)
                for j in range(NG // 2):
                    nc.tensor.transpose(
                        pwt[:, j, :], w_sb[:, h * 8 + j, :], ID_sb)
                nc.vector.tensor_copy(
                    out=wt[:, h * 8:(h + 1) * 8, :], in_=pwt)

            def emit_a_pair(xt, c0):
                """A matmuls for chunks c0, c0+1; shared-weight matmuls
                adjacent."""
                pya = pyp.tile([128, 2, NB], F32, tag="pya")
                pyb = pyp.tile([128, 2, NB], F32, tag="pyb")
                for mm in (0, 1):
                    for kk in (0, 1):
                        for py, c in ((pya, c0), (pyb, c0 + 1)):
                            nc.tensor.matmul(
                                py[:, mm, :], lhsT=A_sb[:, kk, mm, :],
                                rhs=xt[:, kk, c * NB:(c + 1) * NB],
                                start=(kk == 0), stop=False)
                return pya, pyb

            def emit_bm_pair(wt, pya, pyb, c0):
                for mm in (0, 1):
                    for py, c in ((pya, c0), (pyb, c0 + 1)):
                        nc.tensor.matmul(
                            py[:, mm, :],
                            lhsT=Bm_sb[:, mm, :],
                            rhs=wt[0:NL * C, c * GPC:(c + 1) * GPC, :],
                            start=False, stop=True)

            def emit_tanh_sq(py, split=False):
                """split=True halves the ops per kk so downstream logit
                matmuls can start after half the tanh (epilogue tail)."""
                t_sb = tsp.tile([128, 2, NB], BF16, tag="t")
                p2_sb = tsp.tile([128, 2, NB], BF16, tag="p2")
                if split:
                    for kk in (0, 1):
                        nc.scalar.activation(
                            out=t_sb[:, kk, :], in_=py[:, kk, :],
                            func=AF.Tanh)
                    for kk in (0, 1):
                        nc.vector.tensor_tensor(
                            out=p2_sb[:, kk, :], in0=t_sb[:, kk, :],
                            in1=t_sb[:, kk, :], op=ALU.mult)
                else:
                    nc.scalar.activation(
                        out=t_sb.rearrange("p k b -> p (k b)"),
                        in_=py.rearrange("p k b -> p (k b)"),
                        func=AF.Tanh)
                    nc.vector.tensor_tensor(
                        out=p2_sb.rearrange("p k b -> p (k b)"),
                        in0=t_sb.rearrange("p k b -> p (k b)"),
                        in1=t_sb.rearrange("p k b -> p (k b)"),
                        op=ALU.mult)
                return t_sb, p2_sb

            def emit_pl_pair(st, c0):
                """Class-logit matmuls for chunks c0, c0+1, weight-shared."""
                ts, t2s, L_sb = st["ts"], st["t2s"], st["L"]
                pla = plp.tile([C, NB], F32, tag="pla")
                plb = plp.tile([C, NB], F32, tag="plb")
                pls = [pla, plb]
                rhs = {0: ts, 1: t2s}
                first, last = (0, 0), (1, 1)
                for e in (0, 1):
                    for kk in (0, 1):
                        for i, c in enumerate((c0, c0 + 1)):
                            nc.tensor.matmul(
                                pls[i], lhsT=EtEp[:, kk, e, :],
                                rhs=rhs[e][c][:, kk, :],
                                start=(e, kk) == first,
                                stop=(e, kk) == last)
                for i, c in enumerate((c0, c0 + 1)):
                    nc.vector.tensor_copy(
                        out=L_sb[:, c * NB:(c + 1) * NB], in_=pls[i])

            def emit_pl_one(st, c):
                """Single-chunk class-logit matmuls (epilogue tail)."""
                ts, t2s, L_sb = st["ts"], st["t2s"], st["L"]
                plx = plp.tile([C, NB], F32, tag="pla" if c % 2 == 0
                               else "plb")
                rhs = {0: ts, 1: t2s}
                for e in (0, 1):
                    for kk in (0, 1):
                        nc.tensor.matmul(
                            plx, lhsT=EtEp[:, kk, e, :],
                            rhs=rhs[e][c][:, kk, :],
                            start=(e, kk) == (0, 0),
                            stop=(e, kk) == (1, 1))
                nc.vector.tensor_copy(
                    out=L_sb[:, c * NB:(c + 1) * NB], in_=plx)

            def flush_tail(st):
                """BOTH pl pairs + output DMA of the previous superchunk,
                emitted at the top of the next one: ~6us of chain-
                independent PE filler for the softmax-chain latency."""
                emit_pl_pair(st, 0)
                emit_pl_pair(st, 2)
                nc.sync.dma_start(
                    out=outT[:, st["sc"] * SC:(st["sc"] + 1) * SC],
                    in_=st["L"])

            # prologue: prefetch two superchunks of x, scores for sc 0
            xts, pss = [None] * NSC, [None] * NSC
            ws = [None] * NSC
            xts[0] = emit_xt_dma(0)
            xts[1] = emit_xt_dma(1)
            pss[0] = emit_scores(xts[0])
            ws[0] = emit_chain_full(pss[0], False)

            prev = None
            for sc in range(NSC):
                xt, w_sb = xts[sc], ws[sc]
                if prev is not None:
                    flush_tail(prev)
                if sc + 2 < NSC:
                    xts[sc + 2] = emit_xt_dma(sc + 2)
                if sc + 1 < NSC:
                    pss[sc + 1] = emit_scores(xts[sc + 1])

                wt = wtp.tile([NL * C, NG, 128], BF16)
                L_sb = lop.tile([C, SC], F32)
                st = {"sc": sc, "L": L_sb,
                      "ts": [None] * NCH, "t2s": [None] * NCH}

                emit_wt_half(w_sb, wt, 0)
                pya, pyb = emit_a_pair(xt, 0)
                emit_bm_pair(wt, pya, pyb, 0)
                st["ts"][0], st["t2s"][0] = emit_tanh_sq(pya)
                st["ts"][1], st["t2s"][1] = emit_tanh_sq(pyb)
                emit_wt_half(w_sb, wt, 1)
                if sc + 1 < NSC:
                    # next superchunk's chain runs mid-block so the next
                    # boundary's transposes never wait on it
                    ws[sc + 1] = emit_chain_full(pss[sc + 1], False)
                pyc, pyd = emit_a_pair(xt, 2)
                emit_bm_pair(wt, pyc, pyd, 2)
                if sc == NSC - 1:
                    emit_pl_pair(st, 0)
                    st["ts"][2], st["t2s"][2] = emit_tanh_sq(pyc,
                                                            split=True)
                    emit_pl_one(st, 2)
                    nc.sync.dma_start(
                        out=outT[:, sc * SC:sc * SC + 3 * NB],
                        in_=st["L"][:, 0:3 * NB])
                    st["ts"][3], st["t2s"][3] = emit_tanh_sq(pyd,
                                                            split=True)
                    emit_pl_one(st, 3)
                    nc.sync.dma_start(
                        out=outT[:, sc * SC + 3 * NB:(sc + 1) * SC],
                        in_=st["L"][:, 3 * NB:SC])
                else:
                    st["ts"][2], st["t2s"][2] = emit_tanh_sq(pyc)
                    st["ts"][3], st["t2s"][3] = emit_tanh_sq(pyd)
                    prev = st
    nc.finalize()
    return nc


_NC_CACHE = None


def _get_nc():
    global _NC_CACHE
    if _NC_CACHE is None:
        _NC_CACHE = _build_nc()
    return _NC_CACHE


def _run(inputs, trace=False):
    P = _precompute(inputs)
    cst = _device_consts(P)
    x = np.asarray(inputs["x"], np.float32)
    xTb = np.ascontiguousarray(x.astype(bf16).T)      # (256, B)
    nc = _get_nc()
    in_maps = []
    for c in range(NCORES):
        m = dict(cst)
        xc = xTb[:, c * BPC:(c + 1) * BPC]            # (256, BPC)
        xc = xc.reshape(2, 128, NSC, SC).transpose(1, 2, 0, 3)
        m["xT"] = np.ascontiguousarray(xc.reshape(128, NSC * 2 * SC))
        in_maps.append(m)
    res = run_bass_kernel_spmd(nc, in_maps, list(range(NCORES)),
                               trace=trace)
    kap = np.asarray(P["kappa"], np.float32)
    out = np.empty((B, C), np.float32)
    for c in range(NCORES):
        L = res.results[c]["outT"].T + kap[None, :]
        out[c * BPC:(c + 1) * BPC] = 1.0 / (1.0 + np.exp(-L))
    return out, res


def kernel(**inputs):
    out, _ = _run(inputs, trace=False)
    return out
